# revision 31
# baseline (speedup 1.0000x reference)
"""Builder for the DecomposableAttention Trainium2 kernel (v2, bf16).

Layouts (per core, NB batches), packed into blobs to minimize per-dispatch
argument overhead through the axon/PJRT tunnel:

  xblob  [nb, 2, 512, 256] bf16 : slot0 x1t [E,L], slot1 x2t [E,L].
  xblob8 [nb, 2, 512, 256] fp8e4: slot0 x1n [L,E] (flat [512,256]), slot1 x2n
                                  (attention operands; e4m3 for DoubleRow).
  wblob  [3712, 1024]      bf16 : fw1t|fw2t|gw1t|gw2t stacked on rows,
                                  then ident128 (cols 0:128).
  wblob16 [3204, 1024]     fp16 : hw1t|hw2t|finw(4 rows)|ident128.
  bblob  [128, 49]         f32  : six bias tile-sets [128,8] + finb col.

All big matmuls run in bf16 (PE streams bf16 moving rows ~10% faster than
f32r and at half the SBUF/DMA cost); PE accumulates fp32 into PSUM; softmax
math is done on the fp32 PSUM values; biases stay fp32 through the
activation engine.  e1 logits are transposed via fp16 (11-bit mantissa) to
protect softmax accuracy; the tiny aggregate tail runs in fp16.

The attention matmuls (att1T = sent2^T w1a^T etc.) run in fp8e4 DoubleRow
perf mode (2 contraction planes per moving row, ~3x the bf16 row rate);
both operands there are data (attention probabilities and sentence
embeddings), so the e4m3 quantization error is independent per element and
averages out in the position sums.

Schedule: G=2 batches per MLP weight pass; software pipelined so each
group's softmax (DVE/Act) hides under the next group's f-MLP (PE):
  per group bg:  e1+rowmax/exp (bg) | f-MLP(bg+1) | transposes+att (bg)
                 | g-MLP (bg)
"""

import sys

for p in ("/opt/trn_rl_repo", "/root/.axon_site/_ro/trn_rl_repo"):
    if p not in sys.path:
        sys.path.insert(0, p)

import numpy as np
import concourse.bass as bass
import concourse.mybir as mybir
from concourse import bacc
from concourse.tile import TileContext

dt = mybir.dt
AF = mybir.ActivationFunctionType
AX = mybir.AxisListType

B, L, E, H, OUT = 128, 256, 512, 1024, 3
NCORES = 8
NB = B // NCORES          # batches per core
G = 2                     # batches per MLP weight pass
KE = E // 128             # 4  k-tiles over E
KH = H // 128             # 8  k-tiles over H
KL = L // 128             # 2  k-tiles over L
BF = dt.bfloat16
F16 = dt.float16
F8 = dt.float8e4

# wblob (bf16) row offsets
_OFF_FW1, _OFF_FW2 = 0, 512
_OFF_GW1, _OFF_GW2 = 1536, 2560
_OFF_IDB = 3584
_WROWS = 3712
# wblob16 (fp16) row offsets
_OFF_HW1, _OFF_HW2, _OFF_FIN, _OFF_ID16 = 0, 2048, 3072, 3076
_WROWS16 = 3204
# bblob col offsets: fb1 fb2 gb1 gb2 hb1 hb2 | finb
_BCOLS = 49


def build_nc(nb=NB, g=G, repeat=1):
    """repeat>1 wraps the whole per-core program in a hardware loop that
    re-runs the identical computation; used by the timing harness to
    amortize dispatch overhead on-device.  Output is idempotent."""
    assert nb % g == 0
    ngrp = nb // g
    nc = bacc.Bacc("TRN2", target_bir_lowering=False)
    GL = g * L

    xblob = nc.declare_dram_parameter("xblob", [nb, 2, 512, 256], BF,
                                      isOutput=False)
    xblob8 = nc.declare_dram_parameter("xblob8", [nb, 2, 512, 256], F8,
                                       isOutput=False)
    wblob = nc.declare_dram_parameter("wblob", [_WROWS, 1024], BF,
                                      isOutput=False)
    wblob16 = nc.declare_dram_parameter("wblob16", [_WROWS16, 1024], F16,
                                        isOutput=False)
    bblob = nc.declare_dram_parameter("bblob", [128, _BCOLS], dt.float32,
                                      isOutput=False)
    out_d = nc.declare_dram_parameter("out", [4, nb], dt.float32, isOutput=True)

    def wrows(blob, off, rows):
        return blob[off:off + rows, :].rearrange("(k p) m -> p k m", p=128)

    from contextlib import ExitStack
    with TileContext(nc) as tc, \
         tc.tile_pool(name="wpool", bufs=1) as wpool, \
         ExitStack() as rep_ctx:
        if repeat > 1:
            rep_ctx.enter_context(tc.For_i(0, repeat, 1, name="rep"))
        s_allT = wpool.tile([128, 2 * KH, nb], dt.float32)  # aggregate input

        def mm_group(psum, lhs_fn, rhs_fn, nk):
            for k in range(nk):
                nc.tensor.matmul(psum, lhs_fn(k), rhs_fn(k),
                                 start=(k == 0), stop=(k == nk - 1))

        with tc.tile_pool(name="fgw", bufs=1) as fgw, \
             tc.tile_pool(name="xp", bufs=3) as xp, \
             tc.tile_pool(name="xnp", bufs=3) as xnp, \
             tc.tile_pool(name="fp", bufs=2) as fp, \
             tc.tile_pool(name="hidp", bufs=2) as hidp, \
             tc.tile_pool(name="smp", bufs=2) as smp, \
             tc.tile_pool(name="attp", bufs=1) as attp, \
             tc.tile_pool(name="scrp", bufs=3) as scrp, \
             tc.tile_pool(name="ps", bufs=3, space="PSUM") as ps, \
             tc.tile_pool(name="ps_e", bufs=2, space="PSUM") as ps_e, \
             tc.tile_pool(name="ps_t", bufs=2, space="PSUM") as ps_t, \
             tc.tile_pool(name="ps_w", bufs=1, space="PSUM") as ps_w:

            # ---------------- loads ----------------
            xts, xns = [], []

            def load_group(bg):
                x1t = xp.tile([128, KE, GL], BF, name="x1t", tag="x1t")
                x2t = xp.tile([128, KE, GL], BF, name="x2t", tag="x2t")
                x1n = xnp.tile([128, KL * g, E], F8, name="x1n", tag="x1n")
                x2n = xnp.tile([128, KL * g, E], F8, name="x2n", tag="x2n")
                for gi in range(g):
                    b = bg * g + gi
                    nc.sync.dma_start(
                        out=x1t[:, :, gi * L:(gi + 1) * L],
                        in_=xblob[b, 0].rearrange("(k p) l -> p k l", p=128))
                    nc.sync.dma_start(
                        out=x2t[:, :, gi * L:(gi + 1) * L],
                        in_=xblob[b, 1].rearrange("(k p) l -> p k l", p=128))
                    nc.sync.dma_start(
                        out=x1n[:, gi * KL:(gi + 1) * KL, :],
                        in_=xblob8[b, 0].rearrange("(k p a) y -> p k (a y)",
                                                   k=KL, p=128, a=2))
                    nc.sync.dma_start(
                        out=x2n[:, gi * KL:(gi + 1) * KL, :],
                        in_=xblob8[b, 1].rearrange("(k p a) y -> p k (a y)",
                                                   k=KL, p=128, a=2))
                xts.append((x1t, x2t))
                xns.append((x1n, x2n))

            load_group(0)
            fw1_sb = fgw.tile([128, KE, H], BF)
            nc.sync.dma_start(out=fw1_sb, in_=wrows(wblob, _OFF_FW1, 512))
            fw2_sb = fgw.tile([128, KH, H], BF)
            nc.sync.dma_start(out=fw2_sb, in_=wrows(wblob, _OFF_FW2, 1024))
            bias_sb = fgw.tile([128, 4 * KH], dt.float32)
            nc.sync.dma_start(out=bias_sb, in_=bblob[:, 0:4 * KH])
            identB = fgw.tile([128, 128], BF)
            nc.sync.dma_start(out=identB, in_=wblob[_OFF_IDB:_OFF_IDB + 128,
                                                    0:128])
            ident16 = fgw.tile([128, 128], F16)
            nc.sync.dma_start(out=ident16,
                              in_=wblob16[_OFF_ID16:_OFF_ID16 + 128, 0:128])
            gw1_sb = fgw.tile([128, KH, H], BF)
            nc.sync.dma_start(out=gw1_sb, in_=wrows(wblob, _OFF_GW1, 1024))
            gw2_sb = fgw.tile([128, KH, H], BF)
            nc.sync.dma_start(out=gw2_sb, in_=wrows(wblob, _OFF_GW2, 1024))
            fb1_sb = bias_sb[:, 0 * KH:1 * KH]
            fb2_sb = bias_sb[:, 1 * KH:2 * KH]
            gb1_sb = bias_sb[:, 2 * KH:3 * KH]
            gb2_sb = bias_sb[:, 3 * KH:4 * KH]

            # ---------------- stages ----------------
            def stage_F(bg):
                """attend MLP f for both sentences of group bg -> f1t/f2t."""
                x1t, x2t = xts[bg]
                f1t = fp.tile([128, KH, GL], BF, name="f1t", tag="f1t")
                f2t = fp.tile([128, KH, GL], BF, name="f2t", tag="f2t")
                for xt, ft in ((x1t, f1t), (x2t, f2t)):
                    hid = hidp.tile([128, KH, GL], BF, name="f_hid",
                                    tag="f_hid")
                    for m in range(KH):
                        psum = ps.tile([128, GL], dt.float32, name="f_ps",
                                       tag="mlp_ps")
                        mm_group(psum,
                                 lambda k, m=m: fw1_sb[:, k,
                                                       m * 128:(m + 1) * 128],
                                 lambda k: xt[:, k], KE)
                        nc.scalar.activation(out=hid[:, m], in_=psum,
                                             func=AF.Relu,
                                             bias=fb1_sb[:, m:m + 1], scale=1.0)
                    for m in range(KH):
                        psum = ps.tile([128, GL], dt.float32, name="f_ps",
                                       tag="mlp_ps")
                        mm_group(psum,
                                 lambda k, m=m: fw2_sb[:, k,
                                                       m * 128:(m + 1) * 128],
                                 lambda k: hid[:, k], KH)
                        nc.scalar.activation(out=ft[:, m], in_=psum,
                                             func=AF.Relu,
                                             bias=fb2_sb[:, m:m + 1], scale=1.0)
                return f1t, f2t

            def softmax_rows(pe_tiles, w_out):
                """Row softmax (over free dim) of KL psum tiles -> w_out."""
                for im in range(KL):
                    pe = pe_tiles[im]
                    nmax = scrp.tile([128, 1], dt.float32, name="nmax",
                                     tag="sm1")
                    nc.vector.reduce_max(out=nmax, in_=pe, axis=AX.X,
                                         negate=True)
                    ex = scrp.tile([128, L], dt.float32, name="ex", tag="smE")
                    rs = scrp.tile([128, 1], dt.float32, name="rs", tag="sm2")
                    nc.scalar.activation(out=ex, in_=pe, func=AF.Exp,
                                         bias=nmax, scale=1.0, accum_out=rs)
                    rr = scrp.tile([128, 1], dt.float32, name="rr", tag="sm3")
                    nc.vector.reciprocal(out=rr, in_=rs)
                    nc.vector.tensor_scalar_mul(w_out[:, im], ex, rr)

            def stage_E(bg, gi, f1t, f2t):
                """e1 for batch (bg,gi): row softmax -> w1a; fp16 transpose
                -> pet psum; col softmax -> w2a.  Returns (w1a, w2a)."""
                gl = slice(gi * L, (gi + 1) * L)
                e1sb = smp.tile([128, KL, L], F16, name="e1sb", tag="e1sb")
                w1a = smp.tile([128, KL, L], BF, name="w1a", tag="w1a")
                w2a = smp.tile([128, KL, L], BF, name="w2a", tag="w2a")
                pes = []
                for im in range(KL):
                    pe1 = ps_e.tile([128, L], dt.float32, name="pe1",
                                    tag="pe1")
                    mm_group(pe1,
                             lambda k, im=im: f1t[:, k, gi * L + im * 128:
                                                  gi * L + (im + 1) * 128],
                             lambda k: f2t[:, k, gl], KH)
                    nc.vector.tensor_copy(e1sb[:, im], pe1)
                    pes.append(pe1)
                softmax_rows(pes, w1a)
                # fp16 transpose of e1 -> pet (one psum tile, KL slices)
                pet = ps_t.tile([128, KL, L], F16, name="pet", tag="pet")
                for a in range(KL):
                    for bq in range(KL):
                        nc.tensor.transpose(
                            pet[:, a, bq * 128:(bq + 1) * 128],
                            e1sb[:, bq, a * 128:(a + 1) * 128], ident16)
                softmax_rows([pet[:, a] for a in range(KL)], w2a)
                return w1a, w2a

            def stage_TA(bg, gi, w1a, w2a, att1, att2):
                """transpose softmax weights, then attention matmuls for
                batch (bg,gi) -> att1/att2 slices."""
                gl = slice(gi * L, (gi + 1) * L)
                x1n, x2n = xns[bg]
                wt = ps_w.tile([128, 2 * KL, L], BF, name="wt", tag="wt")
                w1at = smp.tile([128, KL, L], F8, name="w1at", tag="w1at")
                w2at = smp.tile([128, KL, L], F8, name="w2at", tag="w2at")
                for src, off in ((w1a, 0), (w2a, KL)):
                    for a in range(KL):
                        for bq in range(KL):
                            nc.tensor.transpose(
                                wt[:, off + a, bq * 128:(bq + 1) * 128],
                                src[:, bq, a * 128:(a + 1) * 128], identB)
                for a in range(KL):
                    nc.scalar.activation(out=w1at[:, a], in_=wt[:, a],
                                         func=AF.Identity)
                for a in range(KL):
                    nc.vector.tensor_copy(w2at[:, a], wt[:, KL + a])
                # att1T[e,i] = sum_j sent2[j,e] w1a[i,j]: one fp8e4 DoubleRow
                # matmul per 128-row e-tile (j = 2 planes x 128 partitions).
                for m in range(KE):
                    pa = ps_e.tile([128, L], dt.float32, name="pa", tag="pe1")
                    nc.tensor.matmul(
                        pa, x2n[:, gi * KL:gi * KL + KL,
                                m * 128:(m + 1) * 128],
                        w1at[:, :, :], start=True, stop=True,
                        perf_mode=mybir.MatmulPerfMode.DoubleRow)
                    # att1 copies ride the Act engine, att2 the DVE: halves
                    # the serial psum-drain chain ahead of the g-MLP start.
                    nc.scalar.activation(out=att1[:, m, gl], in_=pa,
                                         func=AF.Identity)
                    pb = ps_e.tile([128, L], dt.float32, name="pb", tag="pe1")
                    nc.tensor.matmul(
                        pb, x1n[:, gi * KL:gi * KL + KL,
                                m * 128:(m + 1) * 128],
                        w2at[:, :, :], start=True, stop=True,
                        perf_mode=mybir.MatmulPerfMode.DoubleRow)
                    nc.vector.tensor_copy(att2[:, m, gl], pb)

            def stage_G(bg, att1, att2, last=False):
                """compare MLP g for both sentences of group bg; accumulate
                position-sums into s_allT columns.  For the last group the
                sums run on DVE (concurrent with Act relus) so the s_allT
                gate for the aggregate tail closes right after the last g
                matmul; earlier groups keep Act-side accum_out, which stays
                off the DVE queue that feeds the next group's softmax."""
                bs = [bg * g + i for i in range(g)]
                x1t, x2t = xts[bg]
                for att, xt, moff in ((att1, x1t, 0), (att2, x2t, KH)):
                    hid = hidp.tile([128, KH, GL], BF, name="g_hid",
                                    tag="g_hid")
                    for m in range(KH):
                        psum = ps.tile([128, GL], dt.float32, name="g_ps",
                                       tag="mlp_ps")
                        # x-part of the concat first (available immediately),
                        # att-part second; accumulation order is commutative.
                        mm_group(psum,
                                 lambda k, m=m:
                                 gw1_sb[:, KE + k, m * 128:(m + 1) * 128]
                                 if k < KE else
                                 gw1_sb[:, k - KE, m * 128:(m + 1) * 128],
                                 lambda k: xt[:, k] if k < KE
                                 else att[:, k - KE], 2 * KE)
                        nc.scalar.activation(out=hid[:, m], in_=psum,
                                             func=AF.Relu,
                                             bias=gb1_sb[:, m:m + 1], scale=1.0)
                    for m in range(KH):
                        psum = ps.tile([128, GL], dt.float32, name="g_ps",
                                       tag="mlp_ps")
                        mm_group(psum,
                                 lambda k, m=m: gw2_sb[:, k,
                                                       m * 128:(m + 1) * 128],
                                 lambda k: hid[:, k], KH)
                        if last:
                            cmp = scrp.tile([128, GL], BF, name="g_cmp",
                                            tag="g_cmp", bufs=3)
                            nc.scalar.activation(out=cmp, in_=psum,
                                                 func=AF.Relu,
                                                 bias=gb2_sb[:, m:m + 1],
                                                 scale=1.0)
                            for gi in range(g):
                                nc.vector.reduce_sum(
                                    out=s_allT[:, moff + m,
                                               bs[gi]:bs[gi] + 1],
                                    in_=cmp[:, gi * L:(gi + 1) * L],
                                    axis=AX.X)
                        else:
                            for gi in range(g):
                                o = scrp.tile([128, L], dt.float32,
                                              name="g_scr", tag="g_scr",
                                              bufs=3)
                                nc.scalar.activation(
                                    out=o, in_=psum[:, gi * L:(gi + 1) * L],
                                    func=AF.Relu, bias=gb2_sb[:, m:m + 1],
                                    scale=1.0,
                                    accum_out=s_allT[:, moff + m,
                                                     bs[gi]:bs[gi] + 1])

            # ---------------- pipelined main loop ----------------
            if ngrp > 1:
                load_group(1)
            fts = stage_F(0)
            for bg in range(ngrp):
                if bg + 2 < ngrp:
                    load_group(bg + 2)
                f1t, f2t = fts
                sm = [stage_E(bg, gi, f1t, f2t) for gi in range(g)]
                if bg + 1 < ngrp:
                    fts = stage_F(bg + 1)
                att1 = attp.tile([128, KE, GL], BF, name="att1", tag="att1")
                att2 = attp.tile([128, KE, GL], BF, name="att2", tag="att2")
                for gi in range(g):
                    stage_TA(bg, gi, sm[gi][0], sm[gi][1], att1, att2)
                stage_G(bg, att1, att2, last=(bg == ngrp - 1))

        # ---------------- tail: aggregate MLP + final linear ----------------
        with tc.tile_pool(name="tailw", bufs=1) as tw, \
             tc.tile_pool(name="ps_tl", bufs=4, space="PSUM") as ps_tl:
            hw1_sb = tw.tile([128, 2 * KH, H], F16)
            hw2_sb = tw.tile([128, KH, H], F16)
            finw_sb = tw.tile([128, KH, 4], F16)
            nc.sync.dma_start(out=hw1_sb, in_=wrows(wblob16, _OFF_HW1, 2048))
            nc.sync.dma_start(out=hw2_sb, in_=wrows(wblob16, _OFF_HW2, 1024))
            nc.sync.dma_start(
                out=finw_sb,
                in_=wblob16[_OFF_FIN:_OFF_FIN + 4, :].rearrange(
                    "a (c p m) -> p (a c) m", c=2, p=128, m=4))
            hbias_sb = tw.tile([128, 2 * KH], dt.float32)
            nc.sync.dma_start(out=hbias_sb, in_=bblob[:, 4 * KH:6 * KH])
            hb1_sb = hbias_sb[:, 0:KH]
            hb2_sb = hbias_sb[:, KH:2 * KH]
            finb_sb = tw.tile([4, 1], dt.float32)
            nc.sync.dma_start(out=finb_sb, in_=bblob[0:4, 48:49])

            s_r = tw.tile([128, 2 * KH, nb], F16)
            nc.vector.tensor_copy(s_r, s_allT)
            h1a = tw.tile([128, KH, nb], F16)
            for m in range(KH):
                pst = ps_tl.tile([128, nb], dt.float32, name="pst", tag="pst")
                mm_group(pst, lambda k, m=m: hw1_sb[:, k, m * 128:(m + 1) * 128],
                         lambda k: s_r[:, k], 2 * KH)
                nc.scalar.activation(out=h1a[:, m], in_=pst, func=AF.Relu,
                                     bias=hb1_sb[:, m:m + 1], scale=1.0)
            h2a = tw.tile([128, KH, nb], F16)
            for m in range(KH):
                pst = ps_tl.tile([128, nb], dt.float32, name="pst", tag="pst")
                mm_group(pst, lambda k, m=m: hw2_sb[:, k, m * 128:(m + 1) * 128],
                         lambda k: h1a[:, k], KH)
                nc.scalar.activation(out=h2a[:, m], in_=pst, func=AF.Relu,
                                     bias=hb2_sb[:, m:m + 1], scale=1.0)
            pfin = ps_tl.tile([4, nb], dt.float32, name="pfin", tag="pfin")
            mm_group(pfin, lambda k: finw_sb[:, k], lambda k: h2a[:, k], KH)
            out_sb = tw.tile([4, nb], dt.float32)
            nc.scalar.activation(out=out_sb, in_=pfin, func=AF.Identity,
                                 bias=finb_sb, scale=1.0)
            nc.sync.dma_start(out=out_d[:], in_=out_sb)

    nc.finalize()
    return nc


def host_inputs(inputs, nb=NB, cores=NCORES):
    """Build per-core in_maps (blob-packed) from the full problem inputs."""
    s1 = np.ascontiguousarray(inputs["sent1"], dtype=np.float32)[:cores * nb]
    s2 = np.ascontiguousarray(inputs["sent2"], dtype=np.float32)[:cores * nb]
    xblob = np.empty((cores * nb, 2, 512, 256), np.float32)
    xblob[:, 0] = np.swapaxes(s1, 1, 2)                     # x1t [E, L]
    xblob[:, 1] = np.swapaxes(s2, 1, 2)                     # x2t [E, L]
    xblob = xblob.astype(mybir.dt.np(BF)).reshape(cores, nb, 2, 512, 256)
    xblob8 = np.empty((cores * nb, 2, 512, 256), np.float32)
    xblob8[:, 0] = s1.reshape(cores * nb, 512, 256)         # x1n flat view
    xblob8[:, 1] = s2.reshape(cores * nb, 512, 256)         # x2n flat view
    xblob8 = xblob8.astype(mybir.dt.np(F8)).reshape(cores, nb, 2, 512, 256)

    def wt(w, npdt):  # [out, in] -> transposed [in, out]
        return np.ascontiguousarray(
            np.asarray(w, np.float32).T).astype(npdt)

    bfn = mybir.dt.np(BF)
    f16n = mybir.dt.np(F16)
    wblob = np.zeros((_WROWS, 1024), bfn)
    wblob[_OFF_FW1:_OFF_FW1 + 512] = wt(inputs["f_w1"], bfn)
    wblob[_OFF_FW2:_OFF_FW2 + 1024] = wt(inputs["f_w2"], bfn)
    wblob[_OFF_GW1:_OFF_GW1 + 1024] = wt(inputs["g_w1"], bfn)
    wblob[_OFF_GW2:_OFF_GW2 + 1024] = wt(inputs["g_w2"], bfn)
    wblob[_OFF_IDB:_OFF_IDB + 128, 0:128] = np.eye(128, dtype=bfn)

    wblob16 = np.zeros((_WROWS16, 1024), f16n)
    wblob16[_OFF_HW1:_OFF_HW1 + 2048] = wt(inputs["h_w1"], f16n)
    wblob16[_OFF_HW2:_OFF_HW2 + 1024] = wt(inputs["h_w2"], f16n)
    finw = np.zeros((4, H), np.float32)
    finw[:OUT] = np.asarray(inputs["fin_w"], np.float32)
    wblob16[_OFF_FIN:_OFF_FIN + 4] = wt(finw, f16n).reshape(4, 1024)
    wblob16[_OFF_ID16:_OFF_ID16 + 128, 0:128] = np.eye(128, dtype=f16n)

    def bias_tiles(bvec):
        return np.asarray(bvec, np.float32).reshape(KH, 128).T

    bblob = np.zeros((128, _BCOLS), np.float32)
    bblob[:, 0:8] = bias_tiles(inputs["f_b1"])
    bblob[:, 8:16] = bias_tiles(inputs["f_b2"])
    bblob[:, 16:24] = bias_tiles(inputs["g_b1"])
    bblob[:, 24:32] = bias_tiles(inputs["g_b2"])
    bblob[:, 32:40] = bias_tiles(inputs["h_b1"])
    bblob[:, 40:48] = bias_tiles(inputs["h_b2"])
    bblob[0:OUT, 48] = np.asarray(inputs["fin_b"], np.float32)

    return [
        {"xblob": xblob[c], "xblob8": xblob8[c], "wblob": wblob,
         "wblob16": wblob16, "bblob": bblob}
        for c in range(cores)
    ]


def assemble_output(results):
    outs = [res["out"].T[:, :OUT] for res in results]   # [nb, 3] each
    return np.ascontiguousarray(np.concatenate(outs, axis=0), dtype=np.float32)


# ----------------------------------------------------------------------------
# Public entry point: kernel(**inputs) -> [128, 3] float32
# ----------------------------------------------------------------------------
from concourse.bass_utils import run_bass_kernel_spmd

_NC_CACHE = {}


def _get_nc():
    key = (NB, G)
    if key not in _NC_CACHE:
        _NC_CACHE[key] = build_nc(nb=NB, g=G)
    return _NC_CACHE[key]


def kernel(**inputs):
    nc = _get_nc()
    in_maps = host_inputs(inputs, nb=NB, cores=NCORES)

    def run_once():
        res = run_bass_kernel_spmd(nc, in_maps, list(range(NCORES)))
        return assemble_output(res.results)

    # The kernel is bit-deterministic; dispatch twice and require agreement
    # to screen out rare transient per-core transfer/execution glitches.
    a = run_once()
    b = run_once()
    if np.array_equal(a, b):
        return b
    for _ in range(3):
        c = run_once()
        if np.array_equal(c, b) or np.array_equal(c, a):
            return c
        a, b = b, c
    return c
